# revision 59
# baseline (speedup 1.0000x reference)
"""Trainium2 Bass kernel for nn_Attention_40759239639448.

Full-input contract: kernel(**inputs) takes the unsharded inputs and returns
the full [B, T, C] output. Internally shards across 8 NeuronCores: tensor
parallel over heads (2 heads per core, both batches on every core); each core
computes a partial out-projection over its 128 head-channels and the host sums
the 8 partials.

Structure (all data bf16 off-PSUM, host-packed single-DMA input layouts,
one ACT table set for the whole kernel — rsqrt/reciprocal computed as
exp(-ln/2) / exp(-ln) so Ln+Exp+Copy share natural_log_exp_and_others):

  chunk pipeline (per 512-token chunk, 3-stage software pipeline
  proj(i) || post1(i-1) || post2(i-2) so the PE matmul stream never waits on
  the ACT/DVE/GpSimd post-processing chains):
    proj: qkv projection in [d, t] layout, PSUM->SBUF copies.
    post1: RoPE via a PE permutation matmul (trig tables carry the rmsnorm
      weight and the rotate-half sign, so the combine is one full-partition
      add), squares on GpSimd, blockdiag mean-square matmuls packed into one
      PSUM bank via col tile_position, rsqrt via DMA-reshape + Ln/Exp.
    post2: rsqrt broadcast matmul + normalize muls, V transpose to [t, d]
      with a ones column appended (softmax denominator rides the PV matmul).
  attention (per (batch, 512-q-tile)): S^T = k^T q with both heads packed as
    row-halves of the PE array (concurrent), variable-width diagonal tiles
    (S/exp/PV shrink as the causal span shrinks), one exp per k-iter over a
    strided 2-head view, static [128,128] staircase mask, y accumulated in
    PSUM with the denominator row, per-q-tile normalize via DMA-reshaped
    exp(-ln) + gpsimd partition_broadcast, out-projection emitted one q-tile
    late so it fills the next tile's exp latency, bf16 partial outputs.
  overlap scopes: the last two chunks' post-processing (PSUM re-budgeted to
    1+1 banks) is emitted under the first two attention q-tiles (s=2/y=1
    pools), then the remaining attention runs with s=3/y=1; the final q-tile
    normalizes and out-projects per-128-token block interleaved with the
    previous tile's deferred out-projection.
"""
import sys

sys.path.insert(0, "/opt/trn_rl_repo")

import numpy as np

B, T, C, H = 2, 2048, 1024, 16
D = C // H            # 64
NC = 8                # cores
TT = B * T            # 4096 flattened tokens
EPS = 1e-6
ROPE_BASE = 10000.0
CH = 512              # phase-1 chunk width (tokens)
NCH = TT // CH        # 8 chunks
KT = 128              # k-tile rows
QT = 512              # q-tile width
NKT = T // KT         # 16 k-tiles per batch
NQT = T // QT         # 4 q-tiles per batch

_cache = {}


def _build():
    import concourse.bacc as bacc
    import concourse.mybir as mybir
    import concourse.tile as tile

    f32 = mybir.dt.float32
    bf16 = mybir.dt.bfloat16
    AF = mybir.ActivationFunctionType

    # Pin Ln/Exp to the one table set containing both, so the table-load
    # chooser never alternates sets (every ACT op here is Copy/Ln/Exp and
    # natural_log_exp_and_others serves all three with a single load).
    _orig_get_tables = bacc.get_activation_tables

    def _pinned_tables(arch):
        t = _orig_get_tables(arch)
        for name, fns in t.items():
            if name != "natural_log_exp_and_others":
                fns.discard(AF.Ln)
                fns.discard(AF.Exp)
        return t

    bacc.get_activation_tables = _pinned_tables
    try:
        return _build_inner(bacc, mybir, tile, f32, bf16, AF)
    finally:
        bacc.get_activation_tables = _orig_get_tables


def _build_inner(bacc, mybir, tile, f32, bf16, AF):
    nc = bacc.Bacc(None, target_bir_lowering=False)

    # ---- DRAM I/O (host-packed for few, large DMAs) ----
    xR_d = nc.dram_tensor("xR", [128, NCH * 4096], bf16, kind="ExternalInput")
    wpk_d = nc.dram_tensor("wpk", [128, 4 * C], bf16, kind="ExternalInput")
    trig_d = nc.dram_tensor("trig", [128, 4 * T], bf16, kind="ExternalInput")
    misc_d = nc.dram_tensor("misc", [128, 520], bf16, kind="ExternalInput")
    out_d = nc.dram_tensor("out", [TT, C], bf16, kind="ExternalOutput")

    with tile.TileContext(nc) as tc:
        with tc.tile_pool(name="persist", bufs=1) as pp:
            qT = pp.tile([128, TT], bf16, tag="qT")
            kT = pp.tile([128, TT], bf16, tag="kT")
            yT = pp.tile([128, TT], bf16, tag="yT")
            vaug0 = pp.tile([128, B * NKT * 65], bf16, tag="vaug0")
            vaug1 = pp.tile([128, B * NKT * 65], bf16, tag="vaug1")
            wpk = pp.tile([128, 4 * C], bf16, tag="wpk")
            trig = pp.tile([128, 4 * T], bf16, tag="trig")
            misc = pp.tile([128, 520], bf16, tag="misc")
            epsb = pp.tile([64, 1], f32, tag="epsb")
            nc.vector.memset(epsb[:], EPS)

            nc.sync.dma_start(wpk[:, 0:C], wpk_d[:, 0:C])

            wq_sb = wpk[:, 0:C]
            wk_sb = wpk[:, C : 2 * C]
            wv_sb = wpk[:, 2 * C : 3 * C]
            wo_sb = wpk[:, 3 * C : 4 * C]
            cosq = trig[:, 0:T]
            sinq = trig[:, T : 2 * T]
            cosk = trig[:, 2 * T : 3 * T]
            sink = trig[:, 3 * T : 4 * T]
            stair = misc[:, 0:128]
            eperm = misc[:, 128:256]
            ident = misc[:, 256:384]
            bd_sb = misc[:, 384:386]
            e2_sb = misc[0:2, 386:514]

            # ones columns of V_aug
            for va in (vaug0, vaug1):
                nc.vector.memset(
                    va[:].rearrange("p (i f) -> p i f", f=65)[:, :, 64], 1.0
                )

            # ============ phase 1 + overlapped tail / attention ============
            with (
                tc.tile_pool(name="xp", bufs=3) as xp,
                tc.tile_pool(name="scr", bufs=5) as scr,
                tc.tile_pool(name="rsp", bufs=2) as rsp,
                tc.tile_pool(name="rsq", bufs=4) as rsq,
                tc.tile_pool(name="p2sb", bufs=6) as p2,
                tc.tile_pool(name="pp2", bufs=4) as ppool,
                tc.tile_pool(name="dnp", bufs=2) as dnp,
            ):

                xts = {}

                def fetch(ci):
                    if ci >= NCH:
                        return
                    xt = xp.tile([128, 4096], bf16, tag="x")
                    if ci < 2:
                        # split first chunks so the first matmuls can start
                        # as soon as the leading 128KB lands
                        for cc in range(8):
                            nc.sync.dma_start(
                                xt[:, 512 * cc : 512 * cc + 512],
                                xR_d[:, ci * 4096 + 512 * cc :
                                     ci * 4096 + 512 * cc + 512],
                            )
                    else:
                        nc.sync.dma_start(
                            xt[:], xR_d[:, ci * 4096 : (ci + 1) * 4096]
                        )
                    xts[ci] = xt

                def proj(ci, qkvp):
                    t0 = ci * CH
                    xt = xts.pop(ci)
                    q_ps = qkvp.tile([128, CH], f32, tag="qkv")
                    k_ps = qkvp.tile([128, CH], f32, tag="qkv")
                    v_ps = qkvp.tile([128, CH], f32, tag="qkv")
                    for w_sb, o_ps in ((wq_sb, q_ps), (wk_sb, k_ps),
                                       (wv_sb, v_ps)):
                        for cc in range(8):
                            nc.tensor.matmul(
                                o_ps[:], w_sb[:, 128 * cc : 128 * cc + 128],
                                xt[:, 512 * cc : 512 * cc + 512],
                                start=(cc == 0), stop=(cc == 7),
                            )
                    q_sb = scr.tile([128, CH], bf16, tag="qsb")
                    k_sb = scr.tile([128, CH], bf16, tag="qsb")
                    vtmp = scr.tile([128, CH], bf16, tag="vtmp")
                    nc.scalar.copy(q_sb[:], q_ps[:])
                    nc.scalar.copy(k_sb[:], k_ps[:])
                    nc.vector.tensor_copy(vtmp[:], v_ps[:])
                    return (ci, t0, q_sb, k_sb, vtmp)

                def post1(st, msp, scrp):
                    ci, t0, q_sb, k_sb, vtmp = st
                    tt0 = t0 % T
                    tc_q = scr.tile([128, CH], bf16, tag="tc")
                    tc_k = scr.tile([128, CH], bf16, tag="tc")
                    ts_q = scr.tile([128, CH], bf16, tag="ts")
                    ts_k = scr.tile([128, CH], bf16, tag="ts")
                    nc.vector.tensor_mul(
                        tc_q[:], q_sb[:], cosq[:, tt0 : tt0 + CH]
                    )
                    nc.vector.tensor_mul(
                        ts_q[:], q_sb[:], sinq[:, tt0 : tt0 + CH]
                    )
                    nc.vector.tensor_mul(
                        tc_k[:], k_sb[:], cosk[:, tt0 : tt0 + CH]
                    )
                    nc.vector.tensor_mul(
                        ts_k[:], k_sb[:], sink[:, tt0 : tt0 + CH]
                    )
                    tsw_q = scrp.tile([128, CH], f32, tag="scr")
                    tsw_k = scrp.tile([128, CH], f32, tag="scr")
                    nc.tensor.matmul(
                        tsw_q[:], eperm[:], ts_q[:], start=True, stop=True
                    )
                    nc.tensor.matmul(
                        tsw_k[:], eperm[:], ts_k[:], start=True, stop=True
                    )
                    o_q = scr.tile([128, CH], bf16, tag="o_")
                    o_k = scr.tile([128, CH], bf16, tag="o_")
                    nc.vector.tensor_add(o_q[:], tc_q[:], tsw_q[:])
                    nc.vector.tensor_add(o_k[:], tc_k[:], tsw_k[:])

                    sq_q = scr.tile([128, CH], bf16, tag="sq")
                    sq_k = scr.tile([128, CH], bf16, tag="sq")
                    if ci >= NCH - 2:
                        nc.vector.tensor_mul(sq_q[:], o_q[:], o_q[:])
                        nc.vector.tensor_mul(sq_k[:], o_k[:], o_k[:])
                    else:
                        nc.gpsimd.tensor_mul(sq_q[:], o_q[:], o_q[:])
                        nc.gpsimd.tensor_mul(sq_k[:], o_k[:], o_k[:])
                    ms4 = msp.tile([66, CH], f32, tag="ms")
                    nc.tensor.matmul(
                        ms4[0:2, :], bd_sb[:], sq_q[:], start=True, stop=True,
                        tile_position=(0, 0),
                    )
                    nc.tensor.matmul(
                        ms4[64:66, :], bd_sb[:], sq_k[:], start=True,
                        stop=True, tile_position=(0, 64),
                    )
                    # rsqrt = exp(-0.5 * ln(ms/D + eps)): keeps every ACT op in
                    # the natural_log_exp_and_others table set (no reloads)
                    rs4_q = rsq.tile([2, CH], bf16, tag="rs")
                    rs4_k = rsq.tile([2, CH], bf16, tag="rs")
                    if ci >= NCH - 2:
                        # pipeline-drain chunks: direct [2, CH] Ln/Exp chain,
                        # no DMA-reshape hops (latency matters, ACT is idle)
                        lgq = rsp.tile([2, CH], f32, tag="mssbq")
                        lgk = rsp.tile([2, CH], f32, tag="mssbk")
                        nc.scalar.activation(
                            lgq[:], ms4[0:2, :], AF.Ln, scale=1.0 / D,
                            bias=epsb[0:2, :],
                        )
                        nc.scalar.activation(
                            lgk[:], ms4[64:66, :], AF.Ln, scale=1.0 / D,
                            bias=epsb[0:2, :],
                        )
                        nc.scalar.activation(rs4_q[:], lgq[:], AF.Exp, scale=-0.5)
                        nc.scalar.activation(rs4_k[:], lgk[:], AF.Exp, scale=-0.5)
                    else:
                        ms_sq = rsp.tile([2, CH], f32, tag="mssbq")
                        ms_sk = rsp.tile([2, CH], f32, tag="mssbk")
                        nc.scalar.copy(ms_sq[:], ms4[0:2, :])
                        nc.vector.tensor_copy(ms_sk[:], ms4[64:66, :])
                        m2 = rsp.tile([64, 32], f32, tag="m2")
                        nc.sync.dma_start(
                            m2[:, 0:16],
                            ms_sq[:].rearrange("o (p f) -> o p f", p=64),
                        )
                        nc.sync.dma_start(
                            m2[:, 16:32],
                            ms_sk[:].rearrange("o (p f) -> o p f", p=64),
                        )
                        lg2 = rsp.tile([64, 32], f32, tag="st")
                        nc.scalar.activation(
                            lg2[:], m2[:], AF.Ln, scale=1.0 / D, bias=epsb[:],
                        )
                        r2 = rsp.tile([64, 32], bf16, tag="r2")
                        nc.scalar.activation(r2[:], lg2[:], AF.Exp, scale=-0.5)
                        nc.sync.dma_start(
                            rs4_q[:].rearrange("o (p f) -> o p f", p=64),
                            r2[:, 0:16],
                        )
                        nc.sync.dma_start(
                            rs4_k[:].rearrange("o (p f) -> o p f", p=64),
                            r2[:, 16:32],
                        )
                    return (ci, t0, vtmp, o_q, o_k, rs4_q, rs4_k)

                def post2(st, scrp):
                    ci, t0, vtmp, o_q, o_k, rs4_q, rs4_k = st
                    tt0 = t0 % T
                    b = t0 // T
                    rsbc_q = scrp.tile([128, CH], f32, tag="scr")
                    rsbc_k = scrp.tile([128, CH], f32, tag="scr")
                    nc.tensor.matmul(
                        rsbc_q[:], e2_sb[:], rs4_q[:], start=True, stop=True
                    )
                    nc.tensor.matmul(
                        rsbc_k[:], e2_sb[:], rs4_k[:], start=True, stop=True
                    )
                    nc.vector.tensor_mul(qT[:, t0 : t0 + CH], o_q[:], rsbc_q[:])
                    nc.vector.tensor_mul(kT[:, t0 : t0 + CH], o_k[:], rsbc_k[:])

                    for jj in range(4):
                        kti = (tt0 // KT) + jj          # k-tile within batch
                        vt_ps = scrp.tile([128, 128], bf16, tag="scr")
                        nc.tensor.transpose(
                            vt_ps[:], vtmp[:, jj * 128 : jj * 128 + 128],
                            ident[:],
                        )
                        base = (b * NKT + kti) * 65
                        nc.scalar.copy(vaug0[:, base : base + 64], vt_ps[:, 0:64])
                        nc.scalar.copy(vaug1[:, base : base + 64], vt_ps[:, 64:128])

                # ---------- attention pieces (pool-parameterized) ----------
                def outproj_tile(q0, tt, spsp):
                    tg = q0 + tt * 128
                    o_ps = spsp.tile([128, C], f32, tag="sps")
                    nc.tensor.matmul(
                        o_ps[:, 0:512], yT[:, tg : tg + 128],
                        wo_sb[:, 0:512], start=True, stop=True,
                    )
                    nc.tensor.matmul(
                        o_ps[:, 512:1024], yT[:, tg : tg + 128],
                        wo_sb[:, 512:1024], start=True, stop=True,
                    )
                    o_sb = p2.tile([128, C], bf16, tag="osb")
                    if tt == 3:
                        nc.scalar.copy(o_sb[:], o_ps[:])
                    else:
                        nc.vector.tensor_copy(o_sb[:], o_ps[:])
                    nc.sync.dma_start(out_d[tg : tg + 128, :], o_sb[:])

                def attend(b, qi, spsp, yp, fine=False, pre=None):
                    bt = b * T
                    q0 = bt + qi * QT
                    nk = 4 * qi + 4
                    y_ps = yp.tile([65, 2 * QT], f32, tag="y")
                    for ki in range(nk):
                        k0 = bt + ki * KT
                        mi = ki - 4 * qi           # >=0 on the diagonal
                        off = max(0, mi) * KT
                        n = QT - off
                        s_ps = spsp.tile([128, 2 * QT], f32, tag="sps")
                        nc.tensor.matmul(
                            s_ps[:, 0:n],
                            kT[0:64, k0 : k0 + KT],
                            qT[0:64, q0 + off : q0 + QT],
                            start=True, stop=True, tile_position=(0, 0),
                        )
                        nc.tensor.matmul(
                            s_ps[:, QT : QT + n],
                            kT[64:128, k0 : k0 + KT],
                            qT[64:128, q0 + off : q0 + QT],
                            start=True, stop=True, tile_position=(64, 0),
                        )
                        p_sb = ppool.tile([128, 2 * QT], bf16, tag="p")
                        sv = s_ps[:].rearrange("p (h q) -> p h q", h=2)
                        pv = p_sb[:].rearrange("p (h q) -> p h q", h=2)
                        nc.scalar.activation(
                            pv[:, :, 0:n], sv[:, :, 0:n], AF.Exp, scale=0.125,
                        )
                        if mi >= 0:
                            nc.vector.tensor_mul(
                                p_sb[:, 0:KT], p_sb[:, 0:KT], stair[:]
                            )
                            nc.vector.tensor_mul(
                                p_sb[:, QT : QT + KT],
                                p_sb[:, QT : QT + KT], stair[:],
                            )
                        base = (b * NKT + ki) * 65
                        nc.tensor.matmul(
                            y_ps[:, off : off + n],
                            vaug0[:, base : base + 65],
                            p_sb[:, 0:n],
                            start=(ki == 0), stop=(ki == nk - 1),
                        )
                        nc.tensor.matmul(
                            y_ps[:, QT + off : QT + off + n],
                            vaug1[:, base : base + 65],
                            p_sb[:, QT : QT + n],
                            start=(ki == 0), stop=(ki == nk - 1),
                        )

                    # normalize q-tile: den reciprocal + broadcast (non-PE)
                    yraw = p2.tile([65, 2 * QT], bf16, tag="yraw")
                    nc.vector.tensor_copy(yraw[:], y_ps[:])
                    if not fine:
                        den2 = dnp.tile([64, 16], bf16, tag="den")
                        nc.sync.dma_start(
                            den2[:],
                            yraw[64:65, :].rearrange("o (p f) -> o p f", p=64),
                        )
                        lgd = dnp.tile([64, 16], f32, tag="lgd")
                        nc.scalar.activation(lgd[:], den2[:], AF.Ln)
                        rcp2 = dnp.tile([64, 16], bf16, tag="rcp")
                        nc.scalar.activation(rcp2[:], lgd[:], AF.Exp, scale=-1.0)
                        rcb = dnp.tile([1, 2 * QT], bf16, tag="rcb")
                        nc.sync.dma_start(
                            rcb[:].rearrange("o (p f) -> o p f", p=64), rcp2[:],
                        )
                        db0 = dnp.tile([64, QT], bf16, tag="db")
                        db1 = dnp.tile([64, QT], bf16, tag="db")
                        nc.gpsimd.partition_broadcast(db0[:], rcb[0:1, 0:QT])
                        nc.gpsimd.partition_broadcast(
                            db1[:], rcb[0:1, QT : 2 * QT]
                        )
                        nc.vector.tensor_mul(
                            yT[0:64, q0 : q0 + QT], yraw[0:64, 0:QT], db0[:]
                        )
                        nc.vector.tensor_mul(
                            yT[64:128, q0 : q0 + QT],
                            yraw[0:64, QT : 2 * QT], db1[:],
                        )
                        return
                    # fine tail: direct-row reciprocal (no DMA reshape hops),
                    # previous tile's deferred out-projection interleaved to
                    # fill the chain latency
                    lgr = dnp.tile([1, 2 * QT], f32, tag="lgr")
                    nc.scalar.activation(lgr[:], yraw[64:65, :], AF.Ln)
                    rcbf = dnp.tile([1, 2 * QT], bf16, tag="rcb")
                    nc.scalar.activation(rcbf[:], lgr[:], AF.Exp, scale=-1.0)
                    if pre is not None:
                        outproj_tile(pre[0] * T + pre[1] * QT, 0, spsp)
                        outproj_tile(pre[0] * T + pre[1] * QT, 1, spsp)
                    db0 = dnp.tile([64, QT], bf16, tag="db")
                    db1 = dnp.tile([64, QT], bf16, tag="db")
                    nc.gpsimd.partition_broadcast(db0[:], rcbf[0:1, 0:QT])
                    nc.gpsimd.partition_broadcast(db1[:], rcbf[0:1, QT : 2 * QT])
                    nc.vector.tensor_mul(
                        yT[0:64, q0 : q0 + QT], yraw[0:64, 0:QT], db0[:]
                    )
                    nc.vector.tensor_mul(
                        yT[64:128, q0 : q0 + QT],
                        yraw[0:64, QT : 2 * QT], db1[:],
                    )
                    if pre is not None:
                        outproj_tile(pre[0] * T + pre[1] * QT, 2, spsp)
                        outproj_tile(pre[0] * T + pre[1] * QT, 3, spsp)
                    for tt in range(4):
                        outproj_tile(q0, tt, spsp)

                # ---------- scope A: main chunk pipeline ----------
                with (
                    tc.tile_pool(name="ps1", bufs=3, space="PSUM") as qkvp,
                    tc.tile_pool(name="ps1b", bufs=1, space="PSUM") as mspA,
                    tc.tile_pool(name="ps1c", bufs=4, space="PSUM") as scrpA,
                ):
                    fetch(0)
                    nc.sync.dma_start(wpk[:, C : 2 * C], wpk_d[:, C : 2 * C])
                    nc.sync.dma_start(
                        wpk[:, 2 * C : 3 * C], wpk_d[:, 2 * C : 3 * C]
                    )
                    fetch(1)
                    nc.sync.dma_start(
                        wpk[:, 3 * C : 4 * C], wpk_d[:, 3 * C : 4 * C]
                    )
                    nc.sync.dma_start(trig[:], trig_d[:])
                    nc.sync.dma_start(misc[:], misc_d[:])
                    st1 = st2 = None
                    for ci in range(NCH):
                        fetch(ci + 2)
                        nxt = proj(ci, qkvp)
                        n2 = post1(st1, mspA, scrpA) if st1 is not None else None
                        if st2 is not None:
                            post2(st2, scrpA)
                        st1, st2 = nxt, n2
                    # leftovers: post1(c7) and post2(c6), post2(c7) move to
                    # scope B where they overlap the first attention tiles

                # ---------- scope B: phase-1 tail under early attention ----
                with (
                    tc.tile_pool(name="psmB", bufs=1, space="PSUM") as mspB,
                    tc.tile_pool(name="pssB", bufs=1, space="PSUM") as scrpB,
                    tc.tile_pool(name="ps2B", bufs=2, space="PSUM") as spsB,
                    tc.tile_pool(name="ps2yB", bufs=1, space="PSUM") as ypB,
                ):
                    attend(0, 0, spsB, ypB)
                    n7 = post1(st1, mspB, scrpB)
                    post2(st2, scrpB)
                    attend(1, 0, spsB, ypB)
                    post2(n7, scrpB)
                    for tt in range(4):
                        outproj_tile(0 * T + 0 * QT, tt, spsB)

                # ---------- scope C: remaining attention ----------
                with (
                    tc.tile_pool(name="ps2", bufs=3, space="PSUM") as spsC,
                    tc.tile_pool(name="ps2y", bufs=1, space="PSUM") as ypC,
                ):
                    order = [(b, qi) for qi in range(1, NQT) for b in range(B)]
                    pend = (1, 0)
                    for idx in range(len(order) + 1):
                        fine = idx == len(order) - 1
                        if idx < len(order):
                            if fine:
                                attend(*order[idx], spsC, ypC, fine=True,
                                       pre=pend)
                                pend = None
                            else:
                                attend(*order[idx], spsC, ypC)
                        if pend is not None:
                            q0p = pend[0] * T + pend[1] * QT
                            for tt in range(4):
                                outproj_tile(q0p, tt, spsC)
                        pend = (order[idx]
                                if (idx < len(order) and not fine) else None)

    nc.finalize()
    return nc


def _host_prep(x, w_qkv, w_out, q_norm_w, k_norm_w):
    import ml_dtypes

    bf = ml_dtypes.bfloat16
    # xR[p, ci*4096 + cc*512 + j] = x[ci*512 + j, cc*128 + p]
    x3 = np.asarray(x, np.float32).reshape(NCH, CH, 8, 128)  # [ci, j, cc, p]
    xR = np.ascontiguousarray(
        x3.transpose(3, 0, 2, 1).reshape(128, NCH * 4096)
    ).astype(bf)

    j = np.arange(32, dtype=np.float64)
    inv = ROPE_BASE ** (-j / 32.0)
    tt = np.arange(T, dtype=np.float64)
    ang = tt[:, None] * inv[None, :]          # [T, 32]
    cos_t = np.cos(ang)
    sin_t = np.sin(ang)

    def trig_tables(w):
        # cos[p] = cos_t[:, p%32] * w[p%64]
        # sin_pre[p] = sign * sin_t[:, p%32] * w[(p%64+32)%64]
        w = np.asarray(w, dtype=np.float64)
        cosr = np.empty((128, T), np.float32)
        sinr = np.empty((128, T), np.float32)
        for p in range(128):
            jj = p % 32
            r = p % 64
            cosr[p] = cos_t[:, jj] * w[r]
            sgn = 1.0 if r < 32 else -1.0
            sinr[p] = sgn * sin_t[:, jj] * w[(r + 32) % 64]
        return cosr, sinr

    cosq, sinq = trig_tables(q_norm_w)
    cosk, sink = trig_tables(k_norm_w)
    trig = np.concatenate([cosq, sinq, cosk, sink], axis=1).astype(bf)

    kp = np.arange(128)[:, None]
    qq = np.arange(128)[None, :]
    misc = np.zeros((128, 520), np.float32)
    misc[:, 0:128] = (kp <= qq).astype(np.float32)   # staircase
    for p in range(128):                              # 32-block swap perm
        blk = p // 64 * 64
        misc[(p - blk + 32) % 64 + blk, 128 + p] = 1.0
    misc[:, 256:384] = np.eye(128, dtype=np.float32)  # ident
    misc[0:64, 384] = 1.0                             # bd col 0
    misc[64:128, 385] = 1.0                           # bd col 1
    misc[0, 386:450] = 1.0                            # e2 row 0
    misc[1, 450:514] = 1.0                            # e2 row 1
    misc = misc.astype(bf)

    def blockT(w):  # [128, 1024] -> SBUF layout [p, cc*128+j] = w[j, cc*128+p]
        return w.reshape(128, 8, 128).transpose(2, 1, 0).reshape(128, C)

    in_maps = []
    for c in range(NC):
        r0 = 2 * c * 64
        rows = slice(r0, r0 + 128)
        wpk = np.concatenate(
            [
                blockT(w_qkv[rows, :]),
                blockT(w_qkv[C + rows.start : C + rows.stop, :]),
                blockT(w_qkv[2 * C + rows.start : 2 * C + rows.stop, :]),
                np.ascontiguousarray(w_out[:, rows].T),
            ],
            axis=1,
        ).astype(bf)
        in_maps.append({"xR": xR, "trig": trig, "misc": misc, "wpk": wpk})
    return in_maps


def kernel(x, w_qkv, w_out, q_norm_w, k_norm_w, _trace=False, _tmpdir=None):
    from concourse.bass_utils import run_bass_kernel_spmd

    if "nc" not in _cache:
        _cache["nc"] = _build()
    nc = _cache["nc"]

    x = np.asarray(x, dtype=np.float32)
    w_qkv = np.asarray(w_qkv, dtype=np.float32)
    w_out = np.asarray(w_out, dtype=np.float32)
    q_norm_w = np.asarray(q_norm_w, dtype=np.float32)
    k_norm_w = np.asarray(k_norm_w, dtype=np.float32)

    in_maps = _host_prep(x, w_qkv, w_out, q_norm_w, k_norm_w)
    res = run_bass_kernel_spmd(
        nc, in_maps, list(range(NC)), trace=_trace, tmpdir=_tmpdir,
    )
    _cache["last_result"] = res
    parts = np.stack(
        [np.asarray(r["out"], dtype=np.float32) for r in res.results], axis=0
    )
    out = parts.sum(axis=0, dtype=np.float64).astype(np.float32)
    return out.reshape(B, T, C)


# revision 60
# speedup vs baseline: 1.0007x; 1.0007x over previous
"""Trainium2 Bass kernel for nn_Attention_40759239639448.

Full-input contract: kernel(**inputs) takes the unsharded inputs and returns
the full [B, T, C] output. Internally shards across 8 NeuronCores: tensor
parallel over heads (2 heads per core, both batches on every core); each core
computes a partial out-projection over its 128 head-channels and the host sums
the 8 partials.

Structure (all data bf16 off-PSUM, host-packed single-DMA input layouts,
one ACT table set for the whole kernel — rsqrt/reciprocal computed as
exp(-ln/2) / exp(-ln) so Ln+Exp+Copy share natural_log_exp_and_others):

  chunk pipeline (per 512-token chunk, 3-stage software pipeline
  proj(i) || post1(i-1) || post2(i-2) so the PE matmul stream never waits on
  the ACT/DVE/GpSimd post-processing chains):
    proj: qkv projection in [d, t] layout, PSUM->SBUF copies.
    post1: RoPE via a PE permutation matmul (trig tables carry the rmsnorm
      weight and the rotate-half sign, so the combine is one full-partition
      add), squares on GpSimd, blockdiag mean-square matmuls packed into one
      PSUM bank via col tile_position, rsqrt via DMA-reshape + Ln/Exp.
    post2: rsqrt broadcast matmul + normalize muls, V transpose to [t, d]
      with a ones column appended (softmax denominator rides the PV matmul).
  attention (per (batch, 512-q-tile)): S^T = k^T q with both heads packed as
    row-halves of the PE array (concurrent), variable-width diagonal tiles
    (S/exp/PV shrink as the causal span shrinks), one exp per k-iter over a
    strided 2-head view, static [128,128] staircase mask, y accumulated in
    PSUM with the denominator row, per-q-tile normalize via DMA-reshaped
    exp(-ln) + gpsimd partition_broadcast, out-projection emitted one q-tile
    late so it fills the next tile's exp latency, bf16 partial outputs.
  overlap scopes: the last two chunks' post-processing (PSUM re-budgeted to
    1+1 banks) is emitted under the first two attention q-tiles (s=2/y=1
    pools), then the remaining attention runs with s=3/y=1; the final q-tile
    normalizes and out-projects per-128-token block interleaved with the
    previous tile's deferred out-projection.
"""
import sys

sys.path.insert(0, "/opt/trn_rl_repo")

import numpy as np

B, T, C, H = 2, 2048, 1024, 16
D = C // H            # 64
NC = 8                # cores
TT = B * T            # 4096 flattened tokens
EPS = 1e-6
ROPE_BASE = 10000.0
CH = 512              # phase-1 chunk width (tokens)
NCH = TT // CH        # 8 chunks
KT = 128              # k-tile rows
QT = 512              # q-tile width
NKT = T // KT         # 16 k-tiles per batch
NQT = T // QT         # 4 q-tiles per batch

_cache = {}


def _build():
    import concourse.bacc as bacc
    import concourse.mybir as mybir
    import concourse.tile as tile

    f32 = mybir.dt.float32
    bf16 = mybir.dt.bfloat16
    AF = mybir.ActivationFunctionType

    # Pin Ln/Exp to the one table set containing both, so the table-load
    # chooser never alternates sets (every ACT op here is Copy/Ln/Exp and
    # natural_log_exp_and_others serves all three with a single load).
    _orig_get_tables = bacc.get_activation_tables

    def _pinned_tables(arch):
        t = _orig_get_tables(arch)
        for name, fns in t.items():
            if name != "natural_log_exp_and_others":
                fns.discard(AF.Ln)
                fns.discard(AF.Exp)
        return t

    bacc.get_activation_tables = _pinned_tables
    try:
        return _build_inner(bacc, mybir, tile, f32, bf16, AF)
    finally:
        bacc.get_activation_tables = _orig_get_tables


def _build_inner(bacc, mybir, tile, f32, bf16, AF):
    nc = bacc.Bacc(None, target_bir_lowering=False)

    # ---- DRAM I/O (host-packed for few, large DMAs) ----
    xR_d = nc.dram_tensor("xR", [128, NCH * 4096], bf16, kind="ExternalInput")
    wpk_d = nc.dram_tensor("wpk", [128, 4 * C], bf16, kind="ExternalInput")
    trig_d = nc.dram_tensor("trig", [128, 4 * T], bf16, kind="ExternalInput")
    misc_d = nc.dram_tensor("misc", [128, 520], bf16, kind="ExternalInput")
    out_d = nc.dram_tensor("out", [TT, C], bf16, kind="ExternalOutput")

    with tile.TileContext(nc) as tc:
        with tc.tile_pool(name="persist", bufs=1) as pp:
            qT = pp.tile([128, TT], bf16, tag="qT")
            kT = pp.tile([128, TT], bf16, tag="kT")
            yT = pp.tile([128, TT], bf16, tag="yT")
            vaug0 = pp.tile([128, B * NKT * 65], bf16, tag="vaug0")
            vaug1 = pp.tile([128, B * NKT * 65], bf16, tag="vaug1")
            wpk = pp.tile([128, 4 * C], bf16, tag="wpk")
            trig = pp.tile([128, 4 * T], bf16, tag="trig")
            misc = pp.tile([128, 520], bf16, tag="misc")
            epsb = pp.tile([64, 1], f32, tag="epsb")
            nc.vector.memset(epsb[:], EPS)

            nc.sync.dma_start(wpk[:, 0:C], wpk_d[:, 0:C])

            wq_sb = wpk[:, 0:C]
            wk_sb = wpk[:, C : 2 * C]
            wv_sb = wpk[:, 2 * C : 3 * C]
            wo_sb = wpk[:, 3 * C : 4 * C]
            cosq = trig[:, 0:T]
            sinq = trig[:, T : 2 * T]
            cosk = trig[:, 2 * T : 3 * T]
            sink = trig[:, 3 * T : 4 * T]
            stair = misc[:, 0:128]
            eperm = misc[:, 128:256]
            ident = misc[:, 256:384]
            bd_sb = misc[:, 384:386]
            e2_sb = misc[0:2, 386:514]

            # ones columns of V_aug
            for va in (vaug0, vaug1):
                nc.vector.memset(
                    va[:].rearrange("p (i f) -> p i f", f=65)[:, :, 64], 1.0
                )

            # ============ phase 1 + overlapped tail / attention ============
            with (
                tc.tile_pool(name="xp", bufs=3) as xp,
                tc.tile_pool(name="scr", bufs=5) as scr,
                tc.tile_pool(name="rsp", bufs=2) as rsp,
                tc.tile_pool(name="rsq", bufs=4) as rsq,
                tc.tile_pool(name="p2sb", bufs=6) as p2,
                tc.tile_pool(name="pp2", bufs=4) as ppool,
                tc.tile_pool(name="dnp", bufs=2) as dnp,
            ):

                xts = {}

                def fetch(ci):
                    if ci >= NCH:
                        return
                    xt = xp.tile([128, 4096], bf16, tag="x")
                    if ci < 2:
                        # split first chunks so the first matmuls can start
                        # as soon as the leading 128KB lands
                        for cc in range(8):
                            nc.sync.dma_start(
                                xt[:, 512 * cc : 512 * cc + 512],
                                xR_d[:, ci * 4096 + 512 * cc :
                                     ci * 4096 + 512 * cc + 512],
                            )
                    else:
                        nc.sync.dma_start(
                            xt[:], xR_d[:, ci * 4096 : (ci + 1) * 4096]
                        )
                    xts[ci] = xt

                def proj(ci, qkvp):
                    t0 = ci * CH
                    xt = xts.pop(ci)
                    q_ps = qkvp.tile([128, CH], f32, tag="qkv")
                    k_ps = qkvp.tile([128, CH], f32, tag="qkv")
                    v_ps = qkvp.tile([128, CH], f32, tag="qkv")
                    for w_sb, o_ps in ((wq_sb, q_ps), (wk_sb, k_ps),
                                       (wv_sb, v_ps)):
                        for cc in range(8):
                            nc.tensor.matmul(
                                o_ps[:], w_sb[:, 128 * cc : 128 * cc + 128],
                                xt[:, 512 * cc : 512 * cc + 512],
                                start=(cc == 0), stop=(cc == 7),
                            )
                    q_sb = scr.tile([128, CH], bf16, tag="qsb")
                    k_sb = scr.tile([128, CH], bf16, tag="qsb")
                    vtmp = scr.tile([128, CH], bf16, tag="vtmp")
                    nc.scalar.copy(q_sb[:], q_ps[:])
                    nc.scalar.copy(k_sb[:], k_ps[:])
                    nc.vector.tensor_copy(vtmp[:], v_ps[:])
                    return (ci, t0, q_sb, k_sb, vtmp)

                def post1(st, msp, scrp):
                    ci, t0, q_sb, k_sb, vtmp = st
                    tt0 = t0 % T
                    tc_q = scr.tile([128, CH], bf16, tag="tc")
                    tc_k = scr.tile([128, CH], bf16, tag="tc")
                    ts_q = scr.tile([128, CH], bf16, tag="ts")
                    ts_k = scr.tile([128, CH], bf16, tag="ts")
                    nc.vector.tensor_mul(
                        tc_q[:], q_sb[:], cosq[:, tt0 : tt0 + CH]
                    )
                    nc.vector.tensor_mul(
                        ts_q[:], q_sb[:], sinq[:, tt0 : tt0 + CH]
                    )
                    nc.vector.tensor_mul(
                        tc_k[:], k_sb[:], cosk[:, tt0 : tt0 + CH]
                    )
                    nc.vector.tensor_mul(
                        ts_k[:], k_sb[:], sink[:, tt0 : tt0 + CH]
                    )
                    tsw_q = scrp.tile([128, CH], f32, tag="scr")
                    tsw_k = scrp.tile([128, CH], f32, tag="scr")
                    nc.tensor.matmul(
                        tsw_q[:], eperm[:], ts_q[:], start=True, stop=True
                    )
                    nc.tensor.matmul(
                        tsw_k[:], eperm[:], ts_k[:], start=True, stop=True
                    )
                    o_q = scr.tile([128, CH], bf16, tag="o_")
                    o_k = scr.tile([128, CH], bf16, tag="o_")
                    nc.vector.tensor_add(o_q[:], tc_q[:], tsw_q[:])
                    nc.vector.tensor_add(o_k[:], tc_k[:], tsw_k[:])

                    sq_q = scr.tile([128, CH], bf16, tag="sq")
                    sq_k = scr.tile([128, CH], bf16, tag="sq")
                    if ci >= NCH - 2:
                        nc.vector.tensor_mul(sq_q[:], o_q[:], o_q[:])
                        nc.vector.tensor_mul(sq_k[:], o_k[:], o_k[:])
                    else:
                        nc.gpsimd.tensor_mul(sq_q[:], o_q[:], o_q[:])
                        nc.gpsimd.tensor_mul(sq_k[:], o_k[:], o_k[:])
                    ms4 = msp.tile([66, CH], f32, tag="ms")
                    nc.tensor.matmul(
                        ms4[0:2, :], bd_sb[:], sq_q[:], start=True, stop=True,
                        tile_position=(0, 0),
                    )
                    nc.tensor.matmul(
                        ms4[64:66, :], bd_sb[:], sq_k[:], start=True,
                        stop=True, tile_position=(0, 64),
                    )
                    # rsqrt = exp(-0.5 * ln(ms/D + eps)): keeps every ACT op in
                    # the natural_log_exp_and_others table set (no reloads)
                    rs4_q = rsq.tile([2, CH], bf16, tag="rs")
                    rs4_k = rsq.tile([2, CH], bf16, tag="rs")
                    if ci >= NCH - 2:
                        # pipeline-drain chunks: direct [2, CH] Ln/Exp chain,
                        # no DMA-reshape hops (latency matters, ACT is idle)
                        lgq = rsp.tile([2, CH], f32, tag="mssbq")
                        lgk = rsp.tile([2, CH], f32, tag="mssbk")
                        nc.scalar.activation(
                            lgq[:], ms4[0:2, :], AF.Ln, scale=1.0 / D,
                            bias=epsb[0:2, :],
                        )
                        nc.scalar.activation(
                            lgk[:], ms4[64:66, :], AF.Ln, scale=1.0 / D,
                            bias=epsb[0:2, :],
                        )
                        nc.scalar.activation(rs4_q[:], lgq[:], AF.Exp, scale=-0.5)
                        nc.scalar.activation(rs4_k[:], lgk[:], AF.Exp, scale=-0.5)
                    else:
                        ms_sq = rsp.tile([2, CH], f32, tag="mssbq")
                        ms_sk = rsp.tile([2, CH], f32, tag="mssbk")
                        nc.scalar.copy(ms_sq[:], ms4[0:2, :])
                        nc.vector.tensor_copy(ms_sk[:], ms4[64:66, :])
                        m2 = rsp.tile([64, 32], f32, tag="m2")
                        nc.sync.dma_start(
                            m2[:, 0:16],
                            ms_sq[:].rearrange("o (p f) -> o p f", p=64),
                        )
                        nc.sync.dma_start(
                            m2[:, 16:32],
                            ms_sk[:].rearrange("o (p f) -> o p f", p=64),
                        )
                        lg2 = rsp.tile([64, 32], f32, tag="st")
                        nc.scalar.activation(
                            lg2[:], m2[:], AF.Ln, scale=1.0 / D, bias=epsb[:],
                        )
                        r2 = rsp.tile([64, 32], bf16, tag="r2")
                        nc.scalar.activation(r2[:], lg2[:], AF.Exp, scale=-0.5)
                        nc.sync.dma_start(
                            rs4_q[:].rearrange("o (p f) -> o p f", p=64),
                            r2[:, 0:16],
                        )
                        nc.sync.dma_start(
                            rs4_k[:].rearrange("o (p f) -> o p f", p=64),
                            r2[:, 16:32],
                        )
                    return (ci, t0, vtmp, o_q, o_k, rs4_q, rs4_k)

                def post2(st, scrp):
                    ci, t0, vtmp, o_q, o_k, rs4_q, rs4_k = st
                    tt0 = t0 % T
                    b = t0 // T
                    rsbc_q = scrp.tile([128, CH], f32, tag="scr")
                    rsbc_k = scrp.tile([128, CH], f32, tag="scr")
                    nc.tensor.matmul(
                        rsbc_q[:], e2_sb[:], rs4_q[:], start=True, stop=True
                    )
                    nc.tensor.matmul(
                        rsbc_k[:], e2_sb[:], rs4_k[:], start=True, stop=True
                    )
                    nc.vector.tensor_mul(qT[:, t0 : t0 + CH], o_q[:], rsbc_q[:])
                    nc.vector.tensor_mul(kT[:, t0 : t0 + CH], o_k[:], rsbc_k[:])

                    for jj in range(4):
                        kti = (tt0 // KT) + jj          # k-tile within batch
                        vt_ps = scrp.tile([128, 128], bf16, tag="scr")
                        nc.tensor.transpose(
                            vt_ps[:], vtmp[:, jj * 128 : jj * 128 + 128],
                            ident[:],
                        )
                        base = (b * NKT + kti) * 65
                        nc.scalar.copy(vaug0[:, base : base + 64], vt_ps[:, 0:64])
                        nc.scalar.copy(vaug1[:, base : base + 64], vt_ps[:, 64:128])

                # ---------- attention pieces (pool-parameterized) ----------
                def outproj_tile(q0, tt, spsp):
                    tg = q0 + tt * 128
                    o_ps = spsp.tile([128, C], f32, tag="sps")
                    nc.tensor.matmul(
                        o_ps[:, 0:512], yT[:, tg : tg + 128],
                        wo_sb[:, 0:512], start=True, stop=True,
                    )
                    nc.tensor.matmul(
                        o_ps[:, 512:1024], yT[:, tg : tg + 128],
                        wo_sb[:, 512:1024], start=True, stop=True,
                    )
                    o_sb = p2.tile([128, C], bf16, tag="osb")
                    if tt == 3:
                        nc.scalar.copy(o_sb[:], o_ps[:])
                    else:
                        nc.vector.tensor_copy(o_sb[:], o_ps[:])
                    nc.sync.dma_start(out_d[tg : tg + 128, :], o_sb[:])

                def attend(b, qi, spsp, yp, fine=False, pre=None):
                    bt = b * T
                    q0 = bt + qi * QT
                    nk = 4 * qi + 4
                    y_ps = yp.tile([65, 2 * QT], f32, tag="y")
                    for ki in range(nk):
                        k0 = bt + ki * KT
                        mi = ki - 4 * qi           # >=0 on the diagonal
                        off = max(0, mi) * KT
                        n = QT - off
                        s_ps = spsp.tile([128, 2 * QT], f32, tag="sps")
                        nc.tensor.matmul(
                            s_ps[:, 0:n],
                            kT[0:64, k0 : k0 + KT],
                            qT[0:64, q0 + off : q0 + QT],
                            start=True, stop=True, tile_position=(0, 0),
                        )
                        nc.tensor.matmul(
                            s_ps[:, QT : QT + n],
                            kT[64:128, k0 : k0 + KT],
                            qT[64:128, q0 + off : q0 + QT],
                            start=True, stop=True, tile_position=(64, 0),
                        )
                        p_sb = ppool.tile([128, 2 * QT], bf16, tag="p")
                        sv = s_ps[:].rearrange("p (h q) -> p h q", h=2)
                        pv = p_sb[:].rearrange("p (h q) -> p h q", h=2)
                        nc.scalar.activation(
                            pv[:, :, 0:n], sv[:, :, 0:n], AF.Exp, scale=0.125,
                        )
                        if mi >= 0:
                            nc.vector.tensor_mul(
                                p_sb[:, 0:KT], p_sb[:, 0:KT], stair[:]
                            )
                            nc.vector.tensor_mul(
                                p_sb[:, QT : QT + KT],
                                p_sb[:, QT : QT + KT], stair[:],
                            )
                        base = (b * NKT + ki) * 65
                        nc.tensor.matmul(
                            y_ps[:, off : off + n],
                            vaug0[:, base : base + 65],
                            p_sb[:, 0:n],
                            start=(ki == 0), stop=(ki == nk - 1),
                        )
                        nc.tensor.matmul(
                            y_ps[:, QT + off : QT + off + n],
                            vaug1[:, base : base + 65],
                            p_sb[:, QT : QT + n],
                            start=(ki == 0), stop=(ki == nk - 1),
                        )

                    # normalize q-tile: den reciprocal + broadcast (non-PE)
                    yraw = p2.tile([65, 2 * QT], bf16, tag="yraw")
                    nc.vector.tensor_copy(yraw[:], y_ps[:])
                    if not fine:
                        den2 = dnp.tile([64, 16], bf16, tag="den")
                        nc.sync.dma_start(
                            den2[:],
                            yraw[64:65, :].rearrange("o (p f) -> o p f", p=64),
                        )
                        lgd = dnp.tile([64, 16], f32, tag="lgd")
                        nc.scalar.activation(lgd[:], den2[:], AF.Ln)
                        rcp2 = dnp.tile([64, 16], bf16, tag="rcp")
                        nc.scalar.activation(rcp2[:], lgd[:], AF.Exp, scale=-1.0)
                        rcb = dnp.tile([1, 2 * QT], bf16, tag="rcb")
                        nc.sync.dma_start(
                            rcb[:].rearrange("o (p f) -> o p f", p=64), rcp2[:],
                        )
                        db0 = dnp.tile([64, QT], bf16, tag="db")
                        db1 = dnp.tile([64, QT], bf16, tag="db")
                        nc.gpsimd.partition_broadcast(db0[:], rcb[0:1, 0:QT])
                        nc.gpsimd.partition_broadcast(
                            db1[:], rcb[0:1, QT : 2 * QT]
                        )
                        nc.vector.tensor_mul(
                            yT[0:64, q0 : q0 + QT], yraw[0:64, 0:QT], db0[:]
                        )
                        nc.vector.tensor_mul(
                            yT[64:128, q0 : q0 + QT],
                            yraw[0:64, QT : 2 * QT], db1[:],
                        )
                        return
                    # fine tail: direct-row reciprocal (no DMA reshape hops),
                    # previous tile's deferred out-projection interleaved to
                    # fill the chain latency
                    lgr = dnp.tile([1, 2 * QT], f32, tag="lgr")
                    nc.scalar.activation(lgr[:], y_ps[64:65, :], AF.Ln)
                    rcbf = dnp.tile([1, 2 * QT], bf16, tag="rcb")
                    nc.scalar.activation(rcbf[:], lgr[:], AF.Exp, scale=-1.0)
                    if pre is not None:
                        outproj_tile(pre[0] * T + pre[1] * QT, 0, spsp)
                        outproj_tile(pre[0] * T + pre[1] * QT, 1, spsp)
                    # reciprocal broadcast on the (idle) PE: K=1 ones matmul
                    db_ps = spsp.tile([128, 2 * QT], f32, tag="sps")
                    nc.tensor.matmul(
                        db_ps[0:64, 0:QT], e2_sb[0:1, 0:64],
                        rcbf[0:1, 0:QT], start=True, stop=True,
                    )
                    nc.tensor.matmul(
                        db_ps[0:64, QT : 2 * QT], e2_sb[0:1, 0:64],
                        rcbf[0:1, QT : 2 * QT], start=True, stop=True,
                    )
                    nc.vector.tensor_mul(
                        yT[0:64, q0 : q0 + QT], yraw[0:64, 0:QT],
                        db_ps[0:64, 0:QT],
                    )
                    nc.vector.tensor_mul(
                        yT[64:128, q0 : q0 + QT],
                        yraw[0:64, QT : 2 * QT], db_ps[0:64, QT : 2 * QT],
                    )
                    if pre is not None:
                        outproj_tile(pre[0] * T + pre[1] * QT, 2, spsp)
                        outproj_tile(pre[0] * T + pre[1] * QT, 3, spsp)
                    for tt in range(4):
                        outproj_tile(q0, tt, spsp)

                # ---------- scope A: main chunk pipeline ----------
                with (
                    tc.tile_pool(name="ps1", bufs=3, space="PSUM") as qkvp,
                    tc.tile_pool(name="ps1b", bufs=1, space="PSUM") as mspA,
                    tc.tile_pool(name="ps1c", bufs=4, space="PSUM") as scrpA,
                ):
                    fetch(0)
                    nc.sync.dma_start(wpk[:, C : 2 * C], wpk_d[:, C : 2 * C])
                    nc.sync.dma_start(
                        wpk[:, 2 * C : 3 * C], wpk_d[:, 2 * C : 3 * C]
                    )
                    fetch(1)
                    nc.sync.dma_start(
                        wpk[:, 3 * C : 4 * C], wpk_d[:, 3 * C : 4 * C]
                    )
                    nc.sync.dma_start(trig[:], trig_d[:])
                    nc.sync.dma_start(misc[:], misc_d[:])
                    st1 = st2 = None
                    for ci in range(NCH):
                        fetch(ci + 2)
                        nxt = proj(ci, qkvp)
                        n2 = post1(st1, mspA, scrpA) if st1 is not None else None
                        if st2 is not None:
                            post2(st2, scrpA)
                        st1, st2 = nxt, n2
                    # leftovers: post1(c7) and post2(c6), post2(c7) move to
                    # scope B where they overlap the first attention tiles

                # ---------- scope B: phase-1 tail under early attention ----
                with (
                    tc.tile_pool(name="psmB", bufs=1, space="PSUM") as mspB,
                    tc.tile_pool(name="pssB", bufs=1, space="PSUM") as scrpB,
                    tc.tile_pool(name="ps2B", bufs=2, space="PSUM") as spsB,
                    tc.tile_pool(name="ps2yB", bufs=1, space="PSUM") as ypB,
                ):
                    attend(0, 0, spsB, ypB)
                    n7 = post1(st1, mspB, scrpB)
                    post2(st2, scrpB)
                    attend(1, 0, spsB, ypB)
                    post2(n7, scrpB)
                    for tt in range(4):
                        outproj_tile(0 * T + 0 * QT, tt, spsB)

                # ---------- scope C: remaining attention ----------
                with (
                    tc.tile_pool(name="ps2", bufs=3, space="PSUM") as spsC,
                    tc.tile_pool(name="ps2y", bufs=1, space="PSUM") as ypC,
                ):
                    order = [(b, qi) for qi in range(1, NQT) for b in range(B)]
                    pend = (1, 0)
                    for idx in range(len(order) + 1):
                        fine = idx == len(order) - 1
                        if idx < len(order):
                            if fine:
                                attend(*order[idx], spsC, ypC, fine=True,
                                       pre=pend)
                                pend = None
                            else:
                                attend(*order[idx], spsC, ypC)
                        if pend is not None:
                            q0p = pend[0] * T + pend[1] * QT
                            for tt in range(4):
                                outproj_tile(q0p, tt, spsC)
                        pend = (order[idx]
                                if (idx < len(order) and not fine) else None)

    nc.finalize()
    return nc


def _host_prep(x, w_qkv, w_out, q_norm_w, k_norm_w):
    import ml_dtypes

    bf = ml_dtypes.bfloat16
    # xR[p, ci*4096 + cc*512 + j] = x[ci*512 + j, cc*128 + p]
    x3 = np.asarray(x, np.float32).reshape(NCH, CH, 8, 128)  # [ci, j, cc, p]
    xR = np.ascontiguousarray(
        x3.transpose(3, 0, 2, 1).reshape(128, NCH * 4096)
    ).astype(bf)

    j = np.arange(32, dtype=np.float64)
    inv = ROPE_BASE ** (-j / 32.0)
    tt = np.arange(T, dtype=np.float64)
    ang = tt[:, None] * inv[None, :]          # [T, 32]
    cos_t = np.cos(ang)
    sin_t = np.sin(ang)

    def trig_tables(w):
        # cos[p] = cos_t[:, p%32] * w[p%64]
        # sin_pre[p] = sign * sin_t[:, p%32] * w[(p%64+32)%64]
        w = np.asarray(w, dtype=np.float64)
        cosr = np.empty((128, T), np.float32)
        sinr = np.empty((128, T), np.float32)
        for p in range(128):
            jj = p % 32
            r = p % 64
            cosr[p] = cos_t[:, jj] * w[r]
            sgn = 1.0 if r < 32 else -1.0
            sinr[p] = sgn * sin_t[:, jj] * w[(r + 32) % 64]
        return cosr, sinr

    cosq, sinq = trig_tables(q_norm_w)
    cosk, sink = trig_tables(k_norm_w)
    trig = np.concatenate([cosq, sinq, cosk, sink], axis=1).astype(bf)

    kp = np.arange(128)[:, None]
    qq = np.arange(128)[None, :]
    misc = np.zeros((128, 520), np.float32)
    misc[:, 0:128] = (kp <= qq).astype(np.float32)   # staircase
    for p in range(128):                              # 32-block swap perm
        blk = p // 64 * 64
        misc[(p - blk + 32) % 64 + blk, 128 + p] = 1.0
    misc[:, 256:384] = np.eye(128, dtype=np.float32)  # ident
    misc[0:64, 384] = 1.0                             # bd col 0
    misc[64:128, 385] = 1.0                           # bd col 1
    misc[0, 386:450] = 1.0                            # e2 row 0
    misc[1, 450:514] = 1.0                            # e2 row 1
    misc = misc.astype(bf)

    def blockT(w):  # [128, 1024] -> SBUF layout [p, cc*128+j] = w[j, cc*128+p]
        return w.reshape(128, 8, 128).transpose(2, 1, 0).reshape(128, C)

    in_maps = []
    for c in range(NC):
        r0 = 2 * c * 64
        rows = slice(r0, r0 + 128)
        wpk = np.concatenate(
            [
                blockT(w_qkv[rows, :]),
                blockT(w_qkv[C + rows.start : C + rows.stop, :]),
                blockT(w_qkv[2 * C + rows.start : 2 * C + rows.stop, :]),
                np.ascontiguousarray(w_out[:, rows].T),
            ],
            axis=1,
        ).astype(bf)
        in_maps.append({"xR": xR, "trig": trig, "misc": misc, "wpk": wpk})
    return in_maps


def kernel(x, w_qkv, w_out, q_norm_w, k_norm_w, _trace=False, _tmpdir=None):
    from concourse.bass_utils import run_bass_kernel_spmd

    if "nc" not in _cache:
        _cache["nc"] = _build()
    nc = _cache["nc"]

    x = np.asarray(x, dtype=np.float32)
    w_qkv = np.asarray(w_qkv, dtype=np.float32)
    w_out = np.asarray(w_out, dtype=np.float32)
    q_norm_w = np.asarray(q_norm_w, dtype=np.float32)
    k_norm_w = np.asarray(k_norm_w, dtype=np.float32)

    in_maps = _host_prep(x, w_qkv, w_out, q_norm_w, k_norm_w)
    res = run_bass_kernel_spmd(
        nc, in_maps, list(range(NC)), trace=_trace, tmpdir=_tmpdir,
    )
    _cache["last_result"] = res
    parts = np.stack(
        [np.asarray(r["out"], dtype=np.float32) for r in res.results], axis=0
    )
    out = parts.sum(axis=0, dtype=np.float64).astype(np.float32)
    return out.reshape(B, T, C)


# revision 61
# speedup vs baseline: 1.2067x; 1.2058x over previous
"""Trainium2 Bass kernel for nn_Attention_40759239639448.

Full-input contract: kernel(**inputs) takes the unsharded inputs and returns
the full [B, T, C] output. Internally shards across 8 NeuronCores: tensor
parallel over heads (2 heads per core, both batches on every core); each core
computes a partial out-projection over its 128 head-channels and the host sums
the 8 partials.

Structure (all data bf16 off-PSUM, host-packed single-DMA input layouts,
one ACT table set for the whole kernel — rsqrt/reciprocal computed as
exp(-ln/2) / exp(-ln) so Ln+Exp+Copy share natural_log_exp_and_others):

  chunk pipeline (per 512-token chunk, 3-stage software pipeline
  proj(i) || post1(i-1) || post2(i-2) so the PE matmul stream never waits on
  the ACT/DVE/GpSimd post-processing chains):
    proj: qkv projection in [d, t] layout, PSUM->SBUF copies.
    post1: RoPE via a PE permutation matmul (trig tables carry the rmsnorm
      weight and the rotate-half sign, so the combine is one full-partition
      add), squares on GpSimd, blockdiag mean-square matmuls packed into one
      PSUM bank via col tile_position, rsqrt via DMA-reshape + Ln/Exp.
    post2: rsqrt broadcast matmul + normalize muls, V transpose to [t, d]
      with a ones column appended (softmax denominator rides the PV matmul).
  attention (per (batch, 512-q-tile)): S^T = k^T q with both heads packed as
    row-halves of the PE array (concurrent), variable-width diagonal tiles
    (S/exp/PV shrink as the causal span shrinks), one exp per k-iter over a
    strided 2-head view, static [128,128] staircase mask, y accumulated in
    PSUM with the denominator row, per-q-tile normalize via DMA-reshaped
    exp(-ln) + gpsimd partition_broadcast, out-projection emitted one q-tile
    late so it fills the next tile's exp latency, bf16 partial outputs.
  overlap scopes: the last two chunks' post-processing (PSUM re-budgeted to
    1+1 banks) is emitted under the first two attention q-tiles (s=2/y=1
    pools), then the remaining attention runs with s=3/y=1; the final q-tile
    normalizes and out-projects per-128-token block interleaved with the
    previous tile's deferred out-projection.
"""
import sys

sys.path.insert(0, "/opt/trn_rl_repo")

import numpy as np

B, T, C, H = 2, 2048, 1024, 16
D = C // H            # 64
NC = 8                # cores
TT = B * T            # 4096 flattened tokens
EPS = 1e-6
ROPE_BASE = 10000.0
CH = 512              # phase-1 chunk width (tokens)
NCH = TT // CH        # 8 chunks
KT = 128              # k-tile rows
QT = 512              # q-tile width
NKT = T // KT         # 16 k-tiles per batch
NQT = T // QT         # 4 q-tiles per batch

_cache = {}


def _build():
    import concourse.bacc as bacc
    import concourse.mybir as mybir
    import concourse.tile as tile

    f32 = mybir.dt.float32
    bf16 = mybir.dt.bfloat16
    AF = mybir.ActivationFunctionType

    # Pin Ln/Exp to the one table set containing both, so the table-load
    # chooser never alternates sets (every ACT op here is Copy/Ln/Exp and
    # natural_log_exp_and_others serves all three with a single load).
    _orig_get_tables = bacc.get_activation_tables

    def _pinned_tables(arch):
        t = _orig_get_tables(arch)
        for name, fns in t.items():
            if name != "natural_log_exp_and_others":
                fns.discard(AF.Ln)
                fns.discard(AF.Exp)
        return t

    bacc.get_activation_tables = _pinned_tables
    try:
        return _build_inner(bacc, mybir, tile, f32, bf16, AF)
    finally:
        bacc.get_activation_tables = _orig_get_tables


def _build_inner(bacc, mybir, tile, f32, bf16, AF):
    nc = bacc.Bacc(None, target_bir_lowering=False)

    # ---- DRAM I/O (host-packed for few, large DMAs) ----
    xR_d = nc.dram_tensor("xR", [128, NCH * 4096], bf16, kind="ExternalInput")
    wpk_d = nc.dram_tensor("wpk", [128, 4 * C], bf16, kind="ExternalInput")
    trig_d = nc.dram_tensor("trig", [128, 4 * T], bf16, kind="ExternalInput")
    misc_d = nc.dram_tensor("misc", [128, 520], bf16, kind="ExternalInput")
    out_d = nc.dram_tensor("out", [TT, C], bf16, kind="ExternalOutput")

    with tile.TileContext(nc) as tc:
        with tc.tile_pool(name="persist", bufs=1) as pp:
            qT = pp.tile([128, TT], bf16, tag="qT")
            kT = pp.tile([128, TT], bf16, tag="kT")
            yT = pp.tile([128, TT], bf16, tag="yT")
            vaug0 = pp.tile([128, B * NKT * 65], bf16, tag="vaug0")
            vaug1 = pp.tile([128, B * NKT * 65], bf16, tag="vaug1")
            wpk = pp.tile([128, 4 * C], bf16, tag="wpk")
            trig = pp.tile([128, 4 * T], bf16, tag="trig")
            misc = pp.tile([128, 520], bf16, tag="misc")
            epsb = pp.tile([64, 1], f32, tag="epsb")
            nc.vector.memset(epsb[:], EPS)

            nc.sync.dma_start(wpk[:, 0:C], wpk_d[:, 0:C])

            wq_sb = wpk[:, 0:C]
            wk_sb = wpk[:, C : 2 * C]
            wv_sb = wpk[:, 2 * C : 3 * C]
            wo_sb = wpk[:, 3 * C : 4 * C]
            cosq = trig[:, 0:T]
            sinq = trig[:, T : 2 * T]
            cosk = trig[:, 2 * T : 3 * T]
            sink = trig[:, 3 * T : 4 * T]
            stair = misc[:, 0:128]
            eperm = misc[:, 128:256]
            ident = misc[:, 256:384]
            bd_sb = misc[:, 384:386]
            e2_sb = misc[0:2, 386:514]

            # ones columns of V_aug
            for va in (vaug0, vaug1):
                nc.vector.memset(
                    va[:].rearrange("p (i f) -> p i f", f=65)[:, :, 64], 1.0
                )

            # ============ phase 1 + overlapped tail / attention ============
            with (
                tc.tile_pool(name="xp", bufs=3) as xp,
                tc.tile_pool(name="scr", bufs=5) as scr,
                tc.tile_pool(name="rsp", bufs=2) as rsp,
                tc.tile_pool(name="rsq", bufs=4) as rsq,
                tc.tile_pool(name="p2sb", bufs=6) as p2,
                tc.tile_pool(name="pp2", bufs=4) as ppool,
                tc.tile_pool(name="dnp", bufs=2) as dnp,
            ):

                xts = {}

                def fetch(ci):
                    if ci >= NCH:
                        return
                    xt = xp.tile([128, 4096], bf16, tag="x")
                    if ci < 2:
                        # split first chunks so the first matmuls can start
                        # as soon as the leading 128KB lands
                        for cc in range(8):
                            nc.sync.dma_start(
                                xt[:, 512 * cc : 512 * cc + 512],
                                xR_d[:, ci * 4096 + 512 * cc :
                                     ci * 4096 + 512 * cc + 512],
                            )
                    else:
                        nc.sync.dma_start(
                            xt[:], xR_d[:, ci * 4096 : (ci + 1) * 4096]
                        )
                    xts[ci] = xt

                def proj(ci, qkvp):
                    t0 = ci * CH
                    xt = xts.pop(ci)
                    q_ps = qkvp.tile([128, CH], f32, tag="qkv")
                    k_ps = qkvp.tile([128, CH], f32, tag="qkv")
                    v_ps = qkvp.tile([128, CH], f32, tag="qkv")
                    for w_sb, o_ps in ((wq_sb, q_ps), (wk_sb, k_ps),
                                       (wv_sb, v_ps)):
                        for cc in range(8):
                            nc.tensor.matmul(
                                o_ps[:], w_sb[:, 128 * cc : 128 * cc + 128],
                                xt[:, 512 * cc : 512 * cc + 512],
                                start=(cc == 0), stop=(cc == 7),
                            )
                    q_sb = scr.tile([128, CH], bf16, tag="qsb")
                    k_sb = scr.tile([128, CH], bf16, tag="qsb")
                    vtmp = scr.tile([128, CH], bf16, tag="vtmp")
                    nc.scalar.copy(q_sb[:], q_ps[:])
                    nc.scalar.copy(k_sb[:], k_ps[:])
                    nc.vector.tensor_copy(vtmp[:], v_ps[:])
                    return (ci, t0, q_sb, k_sb, vtmp)

                def post1(st, msp, scrp):
                    ci, t0, q_sb, k_sb, vtmp = st
                    tt0 = t0 % T
                    tc_q = scr.tile([128, CH], bf16, tag="tc")
                    tc_k = scr.tile([128, CH], bf16, tag="tc")
                    ts_q = scr.tile([128, CH], bf16, tag="ts")
                    ts_k = scr.tile([128, CH], bf16, tag="ts")
                    nc.vector.tensor_mul(
                        tc_q[:], q_sb[:], cosq[:, tt0 : tt0 + CH]
                    )
                    nc.vector.tensor_mul(
                        ts_q[:], q_sb[:], sinq[:, tt0 : tt0 + CH]
                    )
                    nc.vector.tensor_mul(
                        tc_k[:], k_sb[:], cosk[:, tt0 : tt0 + CH]
                    )
                    nc.vector.tensor_mul(
                        ts_k[:], k_sb[:], sink[:, tt0 : tt0 + CH]
                    )
                    tsw_q = scrp.tile([128, CH], f32, tag="scr")
                    tsw_k = scrp.tile([128, CH], f32, tag="scr")
                    nc.tensor.matmul(
                        tsw_q[:], eperm[:], ts_q[:], start=True, stop=True
                    )
                    nc.tensor.matmul(
                        tsw_k[:], eperm[:], ts_k[:], start=True, stop=True
                    )
                    o_q = scr.tile([128, CH], bf16, tag="o_")
                    o_k = scr.tile([128, CH], bf16, tag="o_")
                    nc.vector.tensor_add(o_q[:], tc_q[:], tsw_q[:])
                    nc.vector.tensor_add(o_k[:], tc_k[:], tsw_k[:])

                    sq_q = scr.tile([128, CH], bf16, tag="sq")
                    sq_k = scr.tile([128, CH], bf16, tag="sq")
                    if ci >= NCH - 2:
                        nc.vector.tensor_mul(sq_q[:], o_q[:], o_q[:])
                        nc.vector.tensor_mul(sq_k[:], o_k[:], o_k[:])
                    else:
                        nc.gpsimd.tensor_mul(sq_q[:], o_q[:], o_q[:])
                        nc.gpsimd.tensor_mul(sq_k[:], o_k[:], o_k[:])
                    ms4 = msp.tile([66, CH], f32, tag="ms")
                    nc.tensor.matmul(
                        ms4[0:2, :], bd_sb[:], sq_q[:], start=True, stop=True,
                        tile_position=(0, 0),
                    )
                    nc.tensor.matmul(
                        ms4[64:66, :], bd_sb[:], sq_k[:], start=True,
                        stop=True, tile_position=(0, 64),
                    )
                    # rsqrt = exp(-0.5 * ln(ms/D + eps)): keeps every ACT op in
                    # the natural_log_exp_and_others table set (no reloads)
                    rs4_q = rsq.tile([2, CH], bf16, tag="rs")
                    rs4_k = rsq.tile([2, CH], bf16, tag="rs")
                    if ci >= NCH - 2:
                        # pipeline-drain chunks: direct [2, CH] Ln/Exp chain,
                        # no DMA-reshape hops (latency matters, ACT is idle)
                        lgq = rsp.tile([2, CH], f32, tag="mssbq")
                        lgk = rsp.tile([2, CH], f32, tag="mssbk")
                        nc.scalar.activation(
                            lgq[:], ms4[0:2, :], AF.Ln, scale=1.0 / D,
                            bias=epsb[0:2, :],
                        )
                        nc.scalar.activation(
                            lgk[:], ms4[64:66, :], AF.Ln, scale=1.0 / D,
                            bias=epsb[0:2, :],
                        )
                        nc.scalar.activation(rs4_q[:], lgq[:], AF.Exp, scale=-0.5)
                        nc.scalar.activation(rs4_k[:], lgk[:], AF.Exp, scale=-0.5)
                    else:
                        ms_sq = rsp.tile([2, CH], f32, tag="mssbq")
                        ms_sk = rsp.tile([2, CH], f32, tag="mssbk")
                        nc.scalar.copy(ms_sq[:], ms4[0:2, :])
                        nc.vector.tensor_copy(ms_sk[:], ms4[64:66, :])
                        m2 = rsp.tile([64, 32], f32, tag="m2")
                        nc.sync.dma_start(
                            m2[:, 0:16],
                            ms_sq[:].rearrange("o (p f) -> o p f", p=64),
                        )
                        nc.sync.dma_start(
                            m2[:, 16:32],
                            ms_sk[:].rearrange("o (p f) -> o p f", p=64),
                        )
                        lg2 = rsp.tile([64, 32], f32, tag="st")
                        nc.scalar.activation(
                            lg2[:], m2[:], AF.Ln, scale=1.0 / D, bias=epsb[:],
                        )
                        r2 = rsp.tile([64, 32], bf16, tag="r2")
                        nc.scalar.activation(r2[:], lg2[:], AF.Exp, scale=-0.5)
                        nc.sync.dma_start(
                            rs4_q[:].rearrange("o (p f) -> o p f", p=64),
                            r2[:, 0:16],
                        )
                        nc.sync.dma_start(
                            rs4_k[:].rearrange("o (p f) -> o p f", p=64),
                            r2[:, 16:32],
                        )
                    return (ci, t0, vtmp, o_q, o_k, rs4_q, rs4_k)

                def post2(st, scrp):
                    ci, t0, vtmp, o_q, o_k, rs4_q, rs4_k = st
                    tt0 = t0 % T
                    b = t0 // T
                    rsbc_q = scrp.tile([128, CH], f32, tag="scr")
                    rsbc_k = scrp.tile([128, CH], f32, tag="scr")
                    nc.tensor.matmul(
                        rsbc_q[:], e2_sb[:], rs4_q[:], start=True, stop=True
                    )
                    nc.tensor.matmul(
                        rsbc_k[:], e2_sb[:], rs4_k[:], start=True, stop=True
                    )
                    nc.vector.tensor_mul(qT[:, t0 : t0 + CH], o_q[:], rsbc_q[:])
                    nc.vector.tensor_mul(kT[:, t0 : t0 + CH], o_k[:], rsbc_k[:])

                    for jj in range(4):
                        kti = (tt0 // KT) + jj          # k-tile within batch
                        vt_ps = scrp.tile([128, 128], bf16, tag="scr")
                        nc.tensor.transpose(
                            vt_ps[:], vtmp[:, jj * 128 : jj * 128 + 128],
                            ident[:],
                        )
                        base = (b * NKT + kti) * 65
                        nc.scalar.copy(vaug0[:, base : base + 64], vt_ps[:, 0:64])
                        nc.scalar.copy(vaug1[:, base : base + 64], vt_ps[:, 64:128])

                # ---------- attention pieces (pool-parameterized) ----------
                def outproj_tile(q0, tt, spsp):
                    tg = q0 + tt * 128
                    o_ps = spsp.tile([128, C], f32, tag="sps")
                    nc.tensor.matmul(
                        o_ps[:, 0:512], yT[:, tg : tg + 128],
                        wo_sb[:, 0:512], start=True, stop=True,
                    )
                    nc.tensor.matmul(
                        o_ps[:, 512:1024], yT[:, tg : tg + 128],
                        wo_sb[:, 512:1024], start=True, stop=True,
                    )
                    o_sb = p2.tile([128, C], bf16, tag="osb")
                    if tt % 2 == 1:
                        nc.scalar.copy(o_sb[:], o_ps[:])
                    else:
                        nc.vector.tensor_copy(o_sb[:], o_ps[:])
                    nc.sync.dma_start(out_d[tg : tg + 128, :], o_sb[:])

                def attend(b, qi, spsp, yp, fine=False, pre=None):
                    bt = b * T
                    q0 = bt + qi * QT
                    nk = 4 * qi + 4
                    y_ps = yp.tile([65, 2 * QT], f32, tag="y")
                    for ki in range(nk):
                        k0 = bt + ki * KT
                        mi = ki - 4 * qi           # >=0 on the diagonal
                        off = max(0, mi) * KT
                        n = QT - off
                        s_ps = spsp.tile([128, 2 * QT], f32, tag="sps")
                        nc.tensor.matmul(
                            s_ps[:, 0:n],
                            kT[0:64, k0 : k0 + KT],
                            qT[0:64, q0 + off : q0 + QT],
                            start=True, stop=True, tile_position=(0, 0),
                        )
                        nc.tensor.matmul(
                            s_ps[:, QT : QT + n],
                            kT[64:128, k0 : k0 + KT],
                            qT[64:128, q0 + off : q0 + QT],
                            start=True, stop=True, tile_position=(64, 0),
                        )
                        p_sb = ppool.tile([128, 2 * QT], bf16, tag="p")
                        sv = s_ps[:].rearrange("p (h q) -> p h q", h=2)
                        pv = p_sb[:].rearrange("p (h q) -> p h q", h=2)
                        nc.scalar.activation(
                            pv[:, :, 0:n], sv[:, :, 0:n], AF.Exp, scale=0.125,
                        )
                        if mi >= 0:
                            nc.vector.tensor_mul(
                                p_sb[:, 0:KT], p_sb[:, 0:KT], stair[:]
                            )
                            nc.vector.tensor_mul(
                                p_sb[:, QT : QT + KT],
                                p_sb[:, QT : QT + KT], stair[:],
                            )
                        base = (b * NKT + ki) * 65
                        nc.tensor.matmul(
                            y_ps[:, off : off + n],
                            vaug0[:, base : base + 65],
                            p_sb[:, 0:n],
                            start=(ki == 0), stop=(ki == nk - 1),
                        )
                        nc.tensor.matmul(
                            y_ps[:, QT + off : QT + off + n],
                            vaug1[:, base : base + 65],
                            p_sb[:, QT : QT + n],
                            start=(ki == 0), stop=(ki == nk - 1),
                        )

                    # normalize q-tile: den reciprocal + broadcast (non-PE)
                    yraw = p2.tile([65, 2 * QT], bf16, tag="yraw")
                    nc.vector.tensor_copy(yraw[:], y_ps[:])
                    if not fine:
                        den2 = dnp.tile([64, 16], bf16, tag="den")
                        nc.sync.dma_start(
                            den2[:],
                            yraw[64:65, :].rearrange("o (p f) -> o p f", p=64),
                        )
                        lgd = dnp.tile([64, 16], f32, tag="lgd")
                        nc.scalar.activation(lgd[:], den2[:], AF.Ln)
                        rcp2 = dnp.tile([64, 16], bf16, tag="rcp")
                        nc.scalar.activation(rcp2[:], lgd[:], AF.Exp, scale=-1.0)
                        rcb = dnp.tile([1, 2 * QT], bf16, tag="rcb")
                        nc.sync.dma_start(
                            rcb[:].rearrange("o (p f) -> o p f", p=64), rcp2[:],
                        )
                        db0 = dnp.tile([64, QT], bf16, tag="db")
                        db1 = dnp.tile([64, QT], bf16, tag="db")
                        nc.gpsimd.partition_broadcast(db0[:], rcb[0:1, 0:QT])
                        nc.gpsimd.partition_broadcast(
                            db1[:], rcb[0:1, QT : 2 * QT]
                        )
                        nc.vector.tensor_mul(
                            yT[0:64, q0 : q0 + QT], yraw[0:64, 0:QT], db0[:]
                        )
                        nc.vector.tensor_mul(
                            yT[64:128, q0 : q0 + QT],
                            yraw[0:64, QT : 2 * QT], db1[:],
                        )
                        return
                    # fine tail: direct-row reciprocal (no DMA reshape hops),
                    # previous tile's deferred out-projection interleaved to
                    # fill the chain latency
                    lgr = dnp.tile([1, 2 * QT], f32, tag="lgr")
                    nc.scalar.activation(lgr[:], y_ps[64:65, :], AF.Ln)
                    rcbf = dnp.tile([1, 2 * QT], bf16, tag="rcb")
                    nc.scalar.activation(rcbf[:], lgr[:], AF.Exp, scale=-1.0)
                    if pre is not None:
                        for ptt in range(4):
                            outproj_tile(pre[0] * T + pre[1] * QT, ptt, spsp)
                    # reciprocal broadcast on the (idle) PE: K=1 ones matmul
                    db_ps = spsp.tile([128, 2 * QT], f32, tag="sps")
                    nc.tensor.matmul(
                        db_ps[0:64, 0:QT], e2_sb[0:1, 0:64],
                        rcbf[0:1, 0:QT], start=True, stop=True,
                    )
                    nc.tensor.matmul(
                        db_ps[0:64, QT : 2 * QT], e2_sb[0:1, 0:64],
                        rcbf[0:1, QT : 2 * QT], start=True, stop=True,
                    )
                    nc.vector.tensor_mul(
                        yT[0:64, q0 : q0 + QT], yraw[0:64, 0:QT],
                        db_ps[0:64, 0:QT],
                    )
                    nc.vector.tensor_mul(
                        yT[64:128, q0 : q0 + QT],
                        yraw[0:64, QT : 2 * QT], db_ps[0:64, QT : 2 * QT],
                    )
                    for tt in range(4):
                        outproj_tile(q0, tt, spsp)

                # ---------- scope A: main chunk pipeline ----------
                with (
                    tc.tile_pool(name="ps1", bufs=3, space="PSUM") as qkvp,
                    tc.tile_pool(name="ps1b", bufs=1, space="PSUM") as mspA,
                    tc.tile_pool(name="ps1c", bufs=4, space="PSUM") as scrpA,
                ):
                    fetch(0)
                    nc.sync.dma_start(wpk[:, C : 2 * C], wpk_d[:, C : 2 * C])
                    nc.sync.dma_start(
                        wpk[:, 2 * C : 3 * C], wpk_d[:, 2 * C : 3 * C]
                    )
                    fetch(1)
                    nc.sync.dma_start(
                        wpk[:, 3 * C : 4 * C], wpk_d[:, 3 * C : 4 * C]
                    )
                    nc.sync.dma_start(trig[:], trig_d[:])
                    nc.sync.dma_start(misc[:], misc_d[:])
                    st1 = st2 = None
                    for ci in range(NCH):
                        fetch(ci + 2)
                        nxt = proj(ci, qkvp)
                        n2 = post1(st1, mspA, scrpA) if st1 is not None else None
                        if st2 is not None:
                            post2(st2, scrpA)
                        st1, st2 = nxt, n2
                    # leftovers: post1(c7) and post2(c6), post2(c7) move to
                    # scope B where they overlap the first attention tiles

                # ---------- scope B: phase-1 tail under early attention ----
                with (
                    tc.tile_pool(name="psmB", bufs=1, space="PSUM") as mspB,
                    tc.tile_pool(name="pssB", bufs=1, space="PSUM") as scrpB,
                    tc.tile_pool(name="ps2B", bufs=2, space="PSUM") as spsB,
                    tc.tile_pool(name="ps2yB", bufs=1, space="PSUM") as ypB,
                ):
                    attend(0, 0, spsB, ypB)
                    n7 = post1(st1, mspB, scrpB)
                    post2(st2, scrpB)
                    attend(1, 0, spsB, ypB)
                    post2(n7, scrpB)
                    for tt in range(4):
                        outproj_tile(0 * T + 0 * QT, tt, spsB)

                # ---------- scope C: remaining attention ----------
                with (
                    tc.tile_pool(name="ps2", bufs=3, space="PSUM") as spsC,
                    tc.tile_pool(name="ps2y", bufs=1, space="PSUM") as ypC,
                ):
                    order = [(b, qi) for qi in range(1, NQT) for b in range(B)]
                    pend = (1, 0)
                    for idx in range(len(order) + 1):
                        fine = idx == len(order) - 1
                        if idx < len(order):
                            if fine:
                                attend(*order[idx], spsC, ypC, fine=True,
                                       pre=pend)
                                pend = None
                            else:
                                attend(*order[idx], spsC, ypC)
                        if pend is not None:
                            q0p = pend[0] * T + pend[1] * QT
                            for tt in range(4):
                                outproj_tile(q0p, tt, spsC)
                        pend = (order[idx]
                                if (idx < len(order) and not fine) else None)

    nc.finalize()
    return nc


def _host_prep(x, w_qkv, w_out, q_norm_w, k_norm_w):
    import ml_dtypes

    bf = ml_dtypes.bfloat16
    # xR[p, ci*4096 + cc*512 + j] = x[ci*512 + j, cc*128 + p]
    x3 = np.asarray(x, np.float32).reshape(NCH, CH, 8, 128)  # [ci, j, cc, p]
    xR = np.ascontiguousarray(
        x3.transpose(3, 0, 2, 1).reshape(128, NCH * 4096)
    ).astype(bf)

    j = np.arange(32, dtype=np.float64)
    inv = ROPE_BASE ** (-j / 32.0)
    tt = np.arange(T, dtype=np.float64)
    ang = tt[:, None] * inv[None, :]          # [T, 32]
    cos_t = np.cos(ang)
    sin_t = np.sin(ang)

    def trig_tables(w):
        # cos[p] = cos_t[:, p%32] * w[p%64]
        # sin_pre[p] = sign * sin_t[:, p%32] * w[(p%64+32)%64]
        w = np.asarray(w, dtype=np.float64)
        cosr = np.empty((128, T), np.float32)
        sinr = np.empty((128, T), np.float32)
        for p in range(128):
            jj = p % 32
            r = p % 64
            cosr[p] = cos_t[:, jj] * w[r]
            sgn = 1.0 if r < 32 else -1.0
            sinr[p] = sgn * sin_t[:, jj] * w[(r + 32) % 64]
        return cosr, sinr

    cosq, sinq = trig_tables(q_norm_w)
    cosk, sink = trig_tables(k_norm_w)
    trig = np.concatenate([cosq, sinq, cosk, sink], axis=1).astype(bf)

    kp = np.arange(128)[:, None]
    qq = np.arange(128)[None, :]
    misc = np.zeros((128, 520), np.float32)
    misc[:, 0:128] = (kp <= qq).astype(np.float32)   # staircase
    for p in range(128):                              # 32-block swap perm
        blk = p // 64 * 64
        misc[(p - blk + 32) % 64 + blk, 128 + p] = 1.0
    misc[:, 256:384] = np.eye(128, dtype=np.float32)  # ident
    misc[0:64, 384] = 1.0                             # bd col 0
    misc[64:128, 385] = 1.0                           # bd col 1
    misc[0, 386:450] = 1.0                            # e2 row 0
    misc[1, 450:514] = 1.0                            # e2 row 1
    misc = misc.astype(bf)

    def blockT(w):  # [128, 1024] -> SBUF layout [p, cc*128+j] = w[j, cc*128+p]
        return w.reshape(128, 8, 128).transpose(2, 1, 0).reshape(128, C)

    in_maps = []
    for c in range(NC):
        r0 = 2 * c * 64
        rows = slice(r0, r0 + 128)
        wpk = np.concatenate(
            [
                blockT(w_qkv[rows, :]),
                blockT(w_qkv[C + rows.start : C + rows.stop, :]),
                blockT(w_qkv[2 * C + rows.start : 2 * C + rows.stop, :]),
                np.ascontiguousarray(w_out[:, rows].T),
            ],
            axis=1,
        ).astype(bf)
        in_maps.append({"xR": xR, "trig": trig, "misc": misc, "wpk": wpk})
    return in_maps


def kernel(x, w_qkv, w_out, q_norm_w, k_norm_w, _trace=False, _tmpdir=None):
    from concourse.bass_utils import run_bass_kernel_spmd

    if "nc" not in _cache:
        _cache["nc"] = _build()
    nc = _cache["nc"]

    x = np.asarray(x, dtype=np.float32)
    w_qkv = np.asarray(w_qkv, dtype=np.float32)
    w_out = np.asarray(w_out, dtype=np.float32)
    q_norm_w = np.asarray(q_norm_w, dtype=np.float32)
    k_norm_w = np.asarray(k_norm_w, dtype=np.float32)

    in_maps = _host_prep(x, w_qkv, w_out, q_norm_w, k_norm_w)
    res = run_bass_kernel_spmd(
        nc, in_maps, list(range(NC)), trace=_trace, tmpdir=_tmpdir,
    )
    _cache["last_result"] = res
    parts = np.stack(
        [np.asarray(r["out"], dtype=np.float32) for r in res.results], axis=0
    )
    out = parts.sum(axis=0, dtype=np.float64).astype(np.float32)
    return out.reshape(B, T, C)
